# revision 1
# baseline (speedup 1.0000x reference)
"""Self-contained Trainium2 Bass kernel for nn_MultiHeadAttention_71528385347884.

Strategy: head tensor-parallel across 8 cores (2 heads/core). Per core:
  - QKV projection with x transposed (feature-major q/k, token-major v)
  - RoPE via host-side A/B weight-column packing (no cross-partition ops)
  - causal attention in [s,t] score layout, softmax without max-subtraction
    (scores are bounded ~|4.5|), denominator via all-ones matmul
  - output projection exploits the reference's scrambled
    transpose(0,2,1,3).reshape(B,T,C): each core produces disjoint output
    rows -> host gather is pure concatenation.
"""

import math
import numpy as np
import ml_dtypes

# ---- problem constants (hardcoded; kernel.py must not read spec/reference) ----
B = 2
T = 2048          # sequence length per batch
C = 2048          # model dim
Dh = 128          # head dim
N_HEAD = 16
N_CORES = 8
H_LOCAL = 2       # heads per core
ROPE_BASE = 10000.0
SCALE = 1.0 / math.sqrt(Dh)

BF16 = ml_dtypes.bfloat16


class Cfg:
    """Size parameters so the same builder runs a small CoreSim config."""

    def __init__(self, B=B, T=T, C=C):
        assert T % 512 == 0 and C % 128 == 0
        self.B = B
        self.T = T
        self.C = C
        self.NCC = C // 128        # contraction chunks for qkv matmuls
        self.BT = B * T
        self.NT = T // 512         # 512-wide t-tiles per batch
        self.GRP = C // Dh         # tokens folded per output row by the reshape
        self.TAU = T // self.GRP   # output rows per (b, h); must be 128
        assert self.TAU == 128
        self.ET = max(1, C // 512)  # 512-wide e-tiles of the output
        self.JQK = 4 * 128         # qA,qB,kA,kB feature blocks
        self.JV = H_LOCAL * 128


FULL = Cfg()


# =====================================================================
# Device program builder
# =====================================================================

def build_nc(cfg: Cfg, debug=False, repeat=1):
    import concourse.bass as bass
    import concourse.mybir as mybir
    import concourse.tile as tile
    from concourse import bacc

    f32 = mybir.dt.float32
    bf16 = mybir.dt.bfloat16
    Exp = mybir.ActivationFunctionType.Exp
    Copy = mybir.ActivationFunctionType.Copy

    nc = bacc.Bacc(None, target_bir_lowering=False, debug=debug)

    xt_d = nc.dram_tensor("xt", [128, cfg.NCC, cfg.BT], bf16, kind="ExternalInput")
    wqk_d = nc.dram_tensor("wqk", [128, cfg.NCC, cfg.JQK], bf16, kind="ExternalInput")
    wv_d = nc.dram_tensor("wv", [128, cfg.NCC, cfg.JV], bf16, kind="ExternalInput")
    wp_d = nc.dram_tensor("wp", [128, cfg.GRP, cfg.C], bf16, kind="ExternalInput")
    cc2_d = nc.dram_tensor("cc2", [128, cfg.T], bf16, kind="ExternalInput")
    spm_d = nc.dram_tensor("spm", [128, cfg.T], bf16, kind="ExternalInput")
    smp_d = nc.dram_tensor("smp", [128, cfg.T], bf16, kind="ExternalInput")
    masks_d = nc.dram_tensor("masks", [128, 4, 1024], bf16, kind="ExternalInput")
    out_d = nc.dram_tensor("out", [cfg.B, H_LOCAL, 128, cfg.C], f32,
                           kind="ExternalOutput")

    with tile.TileContext(nc) as tc:
        with tc.tile_pool(name="persist", bufs=1) as persist:
            # ---- persistent SBUF state ----
            wqk_sb = persist.tile([128, cfg.NCC, cfg.JQK], bf16, name="wqk_sb",
                                  tag="wqk_sb")
            wv_sb = persist.tile([128, cfg.NCC, cfg.JV], bf16, name="wv_sb",
                                 tag="wv_sb")
            cc2_sb = persist.tile([128, cfg.T], bf16, name="cc2_sb", tag="cc2_sb")
            spm_sb = persist.tile([128, cfg.T], bf16, name="spm_sb", tag="spm_sb")
            smp_sb = persist.tile([128, cfg.T], bf16, name="smp_sb", tag="smp_sb")
            masks_sb = persist.tile([128, 4, 1024], bf16, name="masks_sb",
                                    tag="masks_sb")
            ones_sb = persist.tile([128, 128], bf16, name="ones_sb", tag="ones_sb")

            # first weight quarter up front; the rest streams behind the
            # first x slab on the same (FIFO) sync queue.
            wstep = max(1, cfg.NCC // 4)

            def preload_w(q):
                nc.sync.dma_start(wv_sb[:, q:q + wstep, :],
                                  wv_d[:, q:q + wstep, :])
                nc.sync.dma_start(wqk_sb[:, q:q + wstep, :],
                                  wqk_d[:, q:q + wstep, :])

            preload_w(0)
            nc.vector.memset(ones_sb[:], 1.0)

            # per-(b, head-or-tile) persistent tensors; q/k are stored
            # head-contiguous ([dims 0:128 of head h] on partitions) so the
            # score matmuls contract K=128 in one shot.
            qh_sb, kh_sb = {}, {}
            v_sb, vfm_sb, attn_sb = {}, {}, {}
            for b in range(cfg.B):
                for hl in range(H_LOCAL):
                    qh_sb[(b, hl)] = persist.tile([128, cfg.T], bf16,
                                                  name=f"qh_{b}_{hl}",
                                                  tag=f"qh_{b}_{hl}")
                    kh_sb[(b, hl)] = persist.tile([128, cfg.T], bf16,
                                                  name=f"kh_{b}_{hl}",
                                                  tag=f"kh_{b}_{hl}")
                for hl in range(H_LOCAL):
                    v_sb[(b, hl)] = persist.tile(
                        [128, cfg.T // 128, 128], bf16,
                        name=f"v_{b}_{hl}", tag=f"v_{b}_{hl}")
                    vfm_sb[(b, hl)] = persist.tile(
                        [128, cfg.T], bf16,
                        name=f"vf_{b}_{hl}", tag=f"vf_{b}_{hl}")
                    attn_sb[(b, hl)] = persist.tile(
                        [128, cfg.T], bf16,
                        name=f"at_{b}_{hl}", tag=f"at_{b}_{hl}")

            for rep in range(repeat):
                # ========== Phase B: fused QKV projection + RoPE ==========
                # v is computed feature-major (N=512 moving) and flipped to
                # token-major afterwards with one transpose-DMA per head.
                with (
                    tc.tile_pool(name=f"xb_pool{rep}", bufs=4) as xb_pool,
                    tc.tile_pool(name=f"rtmp{rep}", bufs=4) as rtmp,
                    tc.tile_pool(name=f"qkps{rep}", bufs=8, space="PSUM") as qkps,
                ):
                    half = max(1, cfg.NCC // 2)
                    for b in range(cfg.B):
                        for tt in range(cfg.NT):
                            bt0 = b * cfg.T + tt * 512
                            tl = slice(tt * 512, (tt + 1) * 512)
                            pj = [qkps.tile([128, 512], f32, name=f"pj_{b}_{tt}_{j}",
                                            tag="pj") for j in range(6)]
                            xlo = xb_pool.tile([128, half, 512], bf16,
                                               name=f"xbl_{b}_{tt}", tag="xb")
                            xhi = xb_pool.tile([128, half, 512], bf16,
                                               name=f"xbh_{b}_{tt}", tag="xb")
                            qtr = max(1, half // 2)
                            nc.sync.dma_start(xlo[:, 0:qtr, :],
                                              xt_d[:, 0:qtr, bt0:bt0 + 512])
                            nc.sync.dma_start(xlo[:, qtr:half, :],
                                              xt_d[:, qtr:half, bt0:bt0 + 512])
                            nc.gpsimd.dma_start(xhi[:],
                                                xt_d[:, half:cfg.NCC,
                                                     bt0:bt0 + 512])
                            if rep == 0 and b == 0 and tt == 0:
                                for q in range(wstep, cfg.NCC, wstep):
                                    preload_w(q)
                                # tile 0's rope needs only the first 512 trig
                                # columns; keep the startup DMA window small.
                                nc.scalar.dma_start(cc2_sb[:, 0:512],
                                                    cc2_d[:, 0:512])
                                nc.scalar.dma_start(spm_sb[:, 0:512],
                                                    spm_d[:, 0:512])
                                nc.scalar.dma_start(smp_sb[:, 0:512],
                                                    smp_d[:, 0:512])
                            if rep == 0 and b == 0 and tt == min(1, cfg.NT - 1):
                                if cfg.T > 512:
                                    nc.scalar.dma_start(cc2_sb[:, 512:cfg.T],
                                                        cc2_d[:, 512:cfg.T])
                                    nc.scalar.dma_start(spm_sb[:, 512:cfg.T],
                                                        spm_d[:, 512:cfg.T])
                                    nc.scalar.dma_start(smp_sb[:, 512:cfg.T],
                                                        smp_d[:, 512:cfg.T])
                                nc.scalar.dma_start(masks_sb[:], masks_d[:])
                            for ccs in range(cfg.NCC):
                                xb = (xlo if ccs < half else xhi)[:, ccs % half, :]
                                for jc in range(4):
                                    nc.tensor.matmul(
                                        pj[jc][:],
                                        wqk_sb[:, ccs, jc * 128:(jc + 1) * 128],
                                        xb,
                                        start=(ccs == 0), stop=(ccs == cfg.NCC - 1))
                                for hl in range(H_LOCAL):
                                    nc.tensor.matmul(
                                        pj[4 + hl][:],
                                        wv_sb[:, ccs, hl * 128:(hl + 1) * 128],
                                        xb,
                                        start=(ccs == 0), stop=(ccs == cfg.NCC - 1))
                            for hl in range(H_LOCAL):
                                nc.scalar.activation(vfm_sb[(b, hl)][:, tl],
                                                     pj[4 + hl][:], Copy)
                            # rope: rotA = A*C2 + B*S+-,  rotB = B*C2 + A*S-+
                            # rotA rows 0:64 = lo(h0) -> qh0[0:64]  (in place)
                            # rotA rows 64:128 = hi(h1) -> qh1[64:128] (in place)
                            # rotB rows 0:64 = hi(h0) -> qh0[64:128] (DMA move)
                            # rotB rows 64:128 = lo(h1) -> qh1[0:64]  (DMA move)
                            for (Aps, Bps, d0, d1) in (
                                (pj[0], pj[1], qh_sb[(b, 0)], qh_sb[(b, 1)]),
                                (pj[2], pj[3], kh_sb[(b, 0)], kh_sb[(b, 1)]),
                            ):
                                # all four psum-reading muls first: frees the
                                # qkv psum banks ~1.5us earlier per pair, which
                                # is what phase C's first score tiles wait on.
                                m1 = rtmp.tile([128, 512], f32, name="m1", tag="rt")
                                m2 = rtmp.tile([128, 512], f32, name="m2", tag="rt")
                                m3 = rtmp.tile([128, 512], f32, name="m3", tag="rt")
                                m4 = rtmp.tile([128, 512], f32, name="m4", tag="rt")
                                nc.vector.tensor_mul(m1[:], Aps[:], cc2_sb[:, tl])
                                nc.vector.tensor_mul(m2[:], Bps[:], spm_sb[:, tl])
                                nc.vector.tensor_mul(m3[:], Bps[:], cc2_sb[:, tl])
                                nc.vector.tensor_mul(m4[:], Aps[:], smp_sb[:, tl])
                                nc.vector.tensor_add(d0[0:64, tl],
                                                     m1[0:64, :], m2[0:64, :])
                                nc.vector.tensor_add(d1[64:128, tl],
                                                     m1[64:128, :], m2[64:128, :])
                                rb = rtmp.tile([128, 512], bf16, name="rb",
                                               tag="rtb")
                                nc.vector.tensor_add(rb[:], m3[:], m4[:])
                                nc.gpsimd.dma_start(d0[64:128, tl], rb[0:64, :])
                                nc.gpsimd.dma_start(d1[0:64, tl], rb[64:128, :])
                        for hl in range(H_LOCAL):
                            nc.sync.dma_start_transpose(v_sb[(b, hl)][:],
                                                        vfm_sb[(b, hl)][:])

                # ================= Phase C: causal attention ==================
                with (
                    tc.tile_pool(name=f"probs_pool{rep}", bufs=6) as probs_pool,
                    tc.tile_pool(name=f"acc_pool{rep}", bufs=4) as acc_pool,
                    tc.tile_pool(name=f"rec_pool{rep}", bufs=2) as rec_pool,
                    tc.tile_pool(name=f"sps{rep}", bufs=2, space="PSUM") as sps,
                    tc.tile_pool(name=f"ops{rep}", bufs=2, space="PSUM") as ops,
                    tc.tile_pool(name=f"dps{rep}", bufs=2, space="PSUM") as dps,
                ):
                    # prefetch first proj-weight slice during attention
                    wpe_tiles = {}
                    ew = min(512, cfg.C)
                    wpe_tiles[0] = persist.tile([128, cfg.GRP, ew], bf16,
                                                name=f"wpe_0_{rep}", tag="wpe",
                                                bufs=2)
                    nc.scalar.dma_start(wpe_tiles[0][:], wp_d[:, :, 0:ew])
                    for b in range(cfg.B):
                        for tt in range(cfg.NT):
                            tl = slice(tt * 512, (tt + 1) * 512)
                            n_sc = (tt + 1) * 4
                            po = [ops.tile([128, 512], f32, name=f"po_{b}_{tt}_{h}",
                                           tag="po") for h in range(2)]
                            pd = [dps.tile([128, 512], f32,
                                           name=f"pd_{b}_{tt}_{h}", tag="pd")
                                  for h in range(2)]
                            for sc in range(n_sc):
                                sl = slice(sc * 128, (sc + 1) * 128)
                                # both heads' scores in one 2-bank psum tile:
                                # head h lives in columns [h*512, h*512+512)
                                ph = sps.tile([128, 1024], f32,
                                              name=f"ps_{b}_{tt}_{sc}", tag="ps")
                                for h in range(2):
                                    nc.tensor.matmul(
                                        ph[:, h * 512:(h + 1) * 512],
                                        kh_sb[(b, h)][:, sl],
                                        qh_sb[(b, h)][:, tl],
                                        start=True, stop=True)
                                pr = probs_pool.tile([128, 1024], bf16,
                                                     name="pr", tag="pr")
                                nc.scalar.activation(pr[:], ph[:], Exp,
                                                     scale=SCALE)
                                if sc >= tt * 4:  # diagonal block: causal mask
                                    nc.vector.tensor_mul(
                                        pr[:], pr[:],
                                        masks_sb[:, sc - tt * 4, :])
                                for h in range(2):
                                    prh = pr[:, h * 512:(h + 1) * 512]
                                    nc.tensor.matmul(
                                        po[h][:], v_sb[(b, h)][:, sc, :], prh,
                                        start=(sc == 0), stop=(sc == n_sc - 1))
                                    nc.tensor.matmul(
                                        pd[h][:], ones_sb[:], prh,
                                        start=(sc == 0), stop=(sc == n_sc - 1))
                            for h in range(2):
                                rec = rec_pool.tile([128, 512], f32, name=f"rec_{h}",
                                                    tag="rec")
                                nc.vector.reciprocal(rec[:], pd[h][:])
                                nc.vector.tensor_mul(attn_sb[(b, h)][:, tl],
                                                     po[h][:], rec[:])

                # ================= Phase D: output projection =================
                with (
                    tc.tile_pool(name=f"ostg_pool{rep}", bufs=4) as ostg_pool,
                    tc.tile_pool(name=f"pps{rep}", bufs=4, space="PSUM") as pps,
                ):
                    for et in range(cfg.ET):
                        el = slice(et * 512, (et + 1) * 512)
                        if et in wpe_tiles:
                            wpe = wpe_tiles[et]
                        else:
                            wpe = persist.tile([128, cfg.GRP, ew], bf16,
                                               name=f"wpe_{et}_{rep}", tag="wpe",
                                               bufs=2)
                            nc.scalar.dma_start(wpe[:], wp_d[:, :, el])
                        for b in range(cfg.B):
                            for hl in range(H_LOCAL):
                                pp = pps.tile([128, ew], f32,
                                              name=f"pp_{et}_{b}_{hl}", tag="pp")
                                at = attn_sb[(b, hl)]
                                for u in range(cfg.GRP):
                                    nc.tensor.matmul(pp[:], at[:, u::cfg.GRP],
                                                     wpe[:, u, :],
                                                     start=(u == 0),
                                                     stop=(u == cfg.GRP - 1))
                                stg = ostg_pool.tile([128, ew], f32,
                                                     name=f"stg_{et}_{b}_{hl}",
                                                     tag="stg")
                                nc.scalar.activation(stg[:], pp[:], Copy)
                                nc.sync.dma_start(out_d[b, hl, :, el], stg[:])

    nc.compile()
    return nc


# =====================================================================
# Host-side input prep / output gather
# =====================================================================

def _part_major(a2d, ncc):
    """[ncc*128, F] -> [128, ncc, F] with row r = chunk*128 + p."""
    F = a2d.shape[1]
    return np.ascontiguousarray(
        a2d.reshape(ncc, 128, F).transpose(1, 0, 2))


def make_trig(cfg: Cfg):
    pos = np.arange(cfg.T, dtype=np.float64)[None, :]        # [1,T]
    j = np.arange(64, dtype=np.float64)[:, None]             # [64,1]
    inv = ROPE_BASE ** (-2.0 * j / Dh)
    ang = pos * inv                                          # [64,T]
    sin = np.sin(ang).astype(np.float32)
    cos = np.cos(ang).astype(np.float32)
    cc2 = np.concatenate([cos, cos], axis=0).astype(BF16)    # [128,T]
    spm = np.concatenate([-sin, sin], axis=0).astype(BF16)
    smp = np.concatenate([sin, -sin], axis=0).astype(BF16)
    return cc2, spm, smp


def make_masks():
    p = np.arange(128)[:, None]
    jj = np.arange(512)[None, :]
    masks = np.stack([((m * 128 + p) <= jj) for m in range(4)], axis=1)
    masks = np.concatenate([masks, masks], axis=2)           # [128,4,1024]
    return masks.astype(BF16)


def make_in_maps(x, w_qkv, w_proj, cfg: Cfg = FULL, n_cores=N_CORES,
                 n_head=N_HEAD):
    x = np.asarray(x, np.float32)
    w_qkv = np.asarray(w_qkv, np.float32)
    w_proj = np.asarray(w_proj, np.float32)
    Cm = cfg.C

    xT = np.ascontiguousarray(x.reshape(cfg.BT, Cm).T)       # [C, BT]
    xt = _part_major(xT, cfg.NCC).astype(BF16)
    wp = _part_major(w_proj, cfg.GRP).astype(BF16)
    cc2, spm, smp = make_trig(cfg)
    masks = make_masks()

    wq = w_qkv[:, 0:Cm]
    wk = w_qkv[:, Cm:2 * Cm]
    wv_all = w_qkv[:, 2 * Cm:3 * Cm]

    in_maps = []
    for c in range(n_cores):
        h0, h1 = 2 * c, 2 * c + 1
        q0 = wq[:, h0 * 128:(h0 + 1) * 128]
        q1 = wq[:, h1 * 128:(h1 + 1) * 128]
        k0 = wk[:, h0 * 128:(h0 + 1) * 128]
        k1 = wk[:, h1 * 128:(h1 + 1) * 128]
        qA = np.concatenate([q0[:, 0:64], q1[:, 64:128]], axis=1)
        qB = np.concatenate([q0[:, 64:128], q1[:, 0:64]], axis=1)
        kA = np.concatenate([k0[:, 0:64], k1[:, 64:128]], axis=1)
        kB = np.concatenate([k0[:, 64:128], k1[:, 0:64]], axis=1)
        wqk = _part_major(
            np.concatenate([qA, qB, kA, kB], axis=1), cfg.NCC).astype(BF16)
        wv = _part_major(
            np.concatenate([wv_all[:, h0 * 128:(h0 + 1) * 128],
                            wv_all[:, h1 * 128:(h1 + 1) * 128]], axis=1),
            cfg.NCC).astype(BF16)
        in_maps.append(dict(xt=xt, wqk=wqk, wv=wv, wp=wp,
                            cc2=cc2, spm=spm, smp=smp, masks=masks))
    return in_maps


def gather(outs, cfg: Cfg = FULL):
    """outs: per-core [B, H_LOCAL, 128, C] -> full [B, T, C]."""
    rows = np.concatenate(
        [o.reshape(cfg.B, H_LOCAL * 128, cfg.C) for o in outs], axis=1)
    return np.ascontiguousarray(rows.reshape(cfg.B, cfg.T, cfg.C))


# =====================================================================
# Public entry point
# =====================================================================

_NC_CACHE = {}


def get_nc(debug=False):
    key = ("full", debug)
    if key not in _NC_CACHE:
        _NC_CACHE[key] = build_nc(FULL, debug=debug)
    return _NC_CACHE[key]


def kernel(x, w_qkv, w_proj):
    from concourse.bass_utils import run_bass_kernel_spmd
    nc = get_nc()
    in_maps = make_in_maps(x, w_qkv, w_proj)
    res = run_bass_kernel_spmd(nc, in_maps, list(range(N_CORES)))
    return gather([res.results[c]["out"] for c in range(N_CORES)])



# revision 8
# speedup vs baseline: 1.3189x; 1.3189x over previous
"""Self-contained Trainium2 Bass kernel for nn_MultiHeadAttention_71528385347884.

Strategy: head tensor-parallel across 8 cores (2 heads/core). Per core:
  - QKV projection with x transposed (feature-major q/k, token-major v)
  - RoPE via host-side A/B weight-column packing (no cross-partition ops)
  - causal attention in [s,t] score layout, softmax without max-subtraction
    (scores are bounded ~|4.5|), denominator via all-ones matmul
  - output projection exploits the reference's scrambled
    transpose(0,2,1,3).reshape(B,T,C): each core produces disjoint output
    rows -> host gather is pure concatenation.
"""

import math
import numpy as np
import ml_dtypes

# ---- problem constants (hardcoded; kernel.py must not read spec/reference) ----
B = 2
T = 2048          # sequence length per batch
C = 2048          # model dim
Dh = 128          # head dim
N_HEAD = 16
N_CORES = 8
H_LOCAL = 2       # heads per core
ROPE_BASE = 10000.0
SCALE = 1.0 / math.sqrt(Dh)

BF16 = ml_dtypes.bfloat16


class Cfg:
    """Size parameters so the same builder runs a small CoreSim config."""

    def __init__(self, B=B, T=T, C=C):
        assert T % 512 == 0 and C % 128 == 0
        self.B = B
        self.T = T
        self.C = C
        self.NCC = C // 128        # contraction chunks for qkv matmuls
        self.BT = B * T
        self.NT = T // 512         # 512-wide t-tiles per batch
        self.GRP = C // Dh         # tokens folded per output row by the reshape
        self.TAU = T // self.GRP   # output rows per (b, h); must be 128
        assert self.TAU == 128
        self.ET = max(1, C // 512)  # 512-wide e-tiles of the output
        self.JQK = 4 * 128         # qA,qB,kA,kB feature blocks
        self.JV = H_LOCAL * 128


FULL = Cfg()


# =====================================================================
# Device program builder
# =====================================================================

def build_nc(cfg: Cfg, debug=False, repeat=1):
    import concourse.bass as bass
    import concourse.mybir as mybir
    import concourse.tile as tile
    from concourse import bacc

    f32 = mybir.dt.float32
    bf16 = mybir.dt.bfloat16
    Exp = mybir.ActivationFunctionType.Exp
    Copy = mybir.ActivationFunctionType.Copy

    nc = bacc.Bacc(None, target_bir_lowering=False, debug=debug)

    xt_d = nc.dram_tensor("xt", [128, cfg.NCC, cfg.BT], bf16, kind="ExternalInput")
    wqk_d = nc.dram_tensor("wqk", [128, cfg.NCC, cfg.JQK], bf16, kind="ExternalInput")
    wv_d = nc.dram_tensor("wv", [128, cfg.NCC, cfg.JV], bf16, kind="ExternalInput")
    wp_d = nc.dram_tensor("wp", [128, cfg.GRP, cfg.C], bf16, kind="ExternalInput")
    cc2_d = nc.dram_tensor("cc2", [128, cfg.T], bf16, kind="ExternalInput")
    spm_d = nc.dram_tensor("spm", [128, cfg.T], bf16, kind="ExternalInput")
    smp_d = nc.dram_tensor("smp", [128, cfg.T], bf16, kind="ExternalInput")
    out_d = nc.dram_tensor("out", [cfg.B, H_LOCAL, 128, cfg.C], f32,
                           kind="ExternalOutput")

    with tile.TileContext(nc) as tc:
        with tc.tile_pool(name="persist", bufs=1) as persist:
            # ---- persistent SBUF state ----
            wqk_sb = persist.tile([128, cfg.NCC, cfg.JQK], bf16, name="wqk_sb",
                                  tag="wqk_sb")
            wv_sb = persist.tile([128, cfg.NCC, cfg.JV], bf16, name="wv_sb",
                                 tag="wv_sb")
            cc2_sb = persist.tile([128, cfg.T], bf16, name="cc2_sb", tag="cc2_sb")
            spm_sb = persist.tile([128, cfg.T], bf16, name="spm_sb", tag="spm_sb")
            smp_sb = persist.tile([128, cfg.T], bf16, name="smp_sb", tag="smp_sb")
            ones_sb = persist.tile([128, 128], bf16, name="ones_sb", tag="ones_sb")

            # first weight quarter up front; the rest streams behind the
            # first x slab on the same (FIFO) sync queue.
            wstep = max(1, cfg.NCC // 4)

            def preload_w(q):
                nc.sync.dma_start(wv_sb[:, q:q + wstep, :],
                                  wv_d[:, q:q + wstep, :])
                nc.sync.dma_start(wqk_sb[:, q:q + wstep, :],
                                  wqk_d[:, q:q + wstep, :])

            preload_w(0)
            nc.vector.memset(ones_sb[:], 1.0)

            # per-(b, head-or-tile) persistent tensors; q/k are stored
            # head-contiguous ([dims 0:128 of head h] on partitions) so the
            # score matmuls contract K=128 in one shot.
            qh_sb, kh_sb = {}, {}
            v_sb, vfm_sb, attn_sb = {}, {}, {}
            for b in range(cfg.B):
                for hl in range(H_LOCAL):
                    qh_sb[(b, hl)] = persist.tile([128, cfg.T], bf16,
                                                  name=f"qh_{b}_{hl}",
                                                  tag=f"qh_{b}_{hl}")
                    kh_sb[(b, hl)] = persist.tile([128, cfg.T], bf16,
                                                  name=f"kh_{b}_{hl}",
                                                  tag=f"kh_{b}_{hl}")
                for hl in range(H_LOCAL):
                    v_sb[(b, hl)] = persist.tile(
                        [128, cfg.T // 128, 128], bf16,
                        name=f"v_{b}_{hl}", tag=f"v_{b}_{hl}")
                    vfm_sb[(b, hl)] = persist.tile(
                        [128, cfg.T], bf16,
                        name=f"vf_{b}_{hl}", tag=f"vf_{b}_{hl}")
                    attn_sb[(b, hl)] = persist.tile(
                        [128, cfg.T], bf16,
                        name=f"at_{b}_{hl}", tag=f"at_{b}_{hl}")

            for rep in range(repeat):
                # ========== Phase B: fused QKV projection + RoPE ==========
                # v is computed feature-major (N=512 moving) and flipped to
                # token-major afterwards with one transpose-DMA per head.
                with (
                    tc.tile_pool(name=f"xb_pool{rep}", bufs=4) as xb_pool,
                    tc.tile_pool(name=f"rtmp{rep}", bufs=4) as rtmp,
                    tc.tile_pool(name=f"qkps{rep}", bufs=8, space="PSUM") as qkps,
                ):
                    half = max(1, cfg.NCC // 2)
                    for b in range(cfg.B):
                        for tt in range(cfg.NT):
                            bt0 = b * cfg.T + tt * 512
                            tl = slice(tt * 512, (tt + 1) * 512)
                            pj = [qkps.tile([128, 512], f32, name=f"pj_{b}_{tt}_{j}",
                                            tag="pj") for j in range(6)]
                            xlo = xb_pool.tile([128, half, 512], bf16,
                                               name=f"xbl_{b}_{tt}", tag="xb")
                            xhi = xb_pool.tile([128, half, 512], bf16,
                                               name=f"xbh_{b}_{tt}", tag="xb")
                            qtr = max(1, half // 2)
                            nc.sync.dma_start(xlo[:, 0:qtr, :],
                                              xt_d[:, 0:qtr, bt0:bt0 + 512])
                            nc.sync.dma_start(xlo[:, qtr:half, :],
                                              xt_d[:, qtr:half, bt0:bt0 + 512])
                            nc.gpsimd.dma_start(xhi[:],
                                                xt_d[:, half:cfg.NCC,
                                                     bt0:bt0 + 512])
                            if rep == 0 and b == 0 and tt == 0:
                                for q in range(wstep, cfg.NCC, wstep):
                                    preload_w(q)
                                # tile 0's rope needs only the first 512 trig
                                # columns; keep the startup DMA window small.
                                nc.scalar.dma_start(cc2_sb[:, 0:512],
                                                    cc2_d[:, 0:512])
                                nc.scalar.dma_start(spm_sb[:, 0:512],
                                                    spm_d[:, 0:512])
                                nc.scalar.dma_start(smp_sb[:, 0:512],
                                                    smp_d[:, 0:512])
                            if rep == 0 and b == 0 and tt == min(1, cfg.NT - 1):
                                if cfg.T > 512:
                                    nc.scalar.dma_start(cc2_sb[:, 512:cfg.T],
                                                        cc2_d[:, 512:cfg.T])
                                    nc.scalar.dma_start(spm_sb[:, 512:cfg.T],
                                                        spm_d[:, 512:cfg.T])
                                    nc.scalar.dma_start(smp_sb[:, 512:cfg.T],
                                                        smp_d[:, 512:cfg.T])
                            for ccs in range(cfg.NCC):
                                xb = (xlo if ccs < half else xhi)[:, ccs % half, :]
                                for jc in range(4):
                                    nc.tensor.matmul(
                                        pj[jc][:],
                                        wqk_sb[:, ccs, jc * 128:(jc + 1) * 128],
                                        xb,
                                        start=(ccs == 0), stop=(ccs == cfg.NCC - 1))
                                for hl in range(H_LOCAL):
                                    nc.tensor.matmul(
                                        pj[4 + hl][:],
                                        wv_sb[:, ccs, hl * 128:(hl + 1) * 128],
                                        xb,
                                        start=(ccs == 0), stop=(ccs == cfg.NCC - 1))
                            for hl in range(H_LOCAL):
                                nc.scalar.activation(vfm_sb[(b, hl)][:, tl],
                                                     pj[4 + hl][:], Copy)
                            # rope: rotA = A*C2 + B*S+-,  rotB = B*C2 + A*S-+
                            # rotA rows 0:64 = lo(h0) -> qh0[0:64]  (in place)
                            # rotA rows 64:128 = hi(h1) -> qh1[64:128] (in place)
                            # rotB rows 0:64 = hi(h0) -> qh0[64:128] (DMA move)
                            # rotB rows 64:128 = lo(h1) -> qh1[0:64]  (DMA move)
                            for (Aps, Bps, d0, d1) in (
                                (pj[0], pj[1], qh_sb[(b, 0)], qh_sb[(b, 1)]),
                                (pj[2], pj[3], kh_sb[(b, 0)], kh_sb[(b, 1)]),
                            ):
                                # all four psum-reading muls first: frees the
                                # qkv psum banks ~1.5us earlier per pair, which
                                # is what phase C's first score tiles wait on.
                                m1 = rtmp.tile([128, 512], f32, name="m1", tag="rt")
                                m2 = rtmp.tile([128, 512], f32, name="m2", tag="rt")
                                m3 = rtmp.tile([128, 512], f32, name="m3", tag="rt")
                                m4 = rtmp.tile([128, 512], f32, name="m4", tag="rt")
                                nc.vector.tensor_mul(m1[:], Aps[:], cc2_sb[:, tl])
                                nc.vector.tensor_mul(m2[:], Bps[:], spm_sb[:, tl])
                                nc.vector.tensor_mul(m3[:], Bps[:], cc2_sb[:, tl])
                                nc.vector.tensor_mul(m4[:], Aps[:], smp_sb[:, tl])
                                nc.vector.tensor_add(d0[0:64, tl],
                                                     m1[0:64, :], m2[0:64, :])
                                nc.vector.tensor_add(d1[64:128, tl],
                                                     m1[64:128, :], m2[64:128, :])
                                rb = rtmp.tile([128, 512], bf16, name="rb",
                                               tag="rtb")
                                nc.vector.tensor_add(rb[:], m3[:], m4[:])
                                nc.gpsimd.dma_start(d0[64:128, tl], rb[0:64, :])
                                nc.gpsimd.dma_start(d1[0:64, tl], rb[64:128, :])
                        for hl in range(H_LOCAL):
                            nc.sync.dma_start_transpose(v_sb[(b, hl)][:],
                                                        vfm_sb[(b, hl)][:])

                # ================= Phase C: causal attention ==================
                # Scores land in [s=partition, t=free] layout, per-head halves
                # of a [128, 2, 512] psum tile. Diagonal 512x512 super-blocks
                # trim the score matmul + exp to the causally-valid t-tail;
                # the 128-wide diagonal triangle (plus the stale region left
                # of it) is zeroed by an affine_select on the Pool engine.
                # The softmax denominator never touches PE per-block: probs
                # accumulate across s-blocks on DVE (bf16), with one final
                # ones-matmul per (b, tt) to contract the 128 partitions.
                with (
                    tc.tile_pool(name=f"probs_pool{rep}", bufs=6) as probs_pool,
                    tc.tile_pool(name=f"dacc_pool{rep}", bufs=3) as dacc_pool,
                    tc.tile_pool(name=f"rec_pool{rep}", bufs=2) as rec_pool,
                    tc.tile_pool(name=f"sps{rep}", bufs=2, space="PSUM") as sps,
                    tc.tile_pool(name=f"ops{rep}", bufs=4, space="PSUM") as ops,
                ):
                    # prefetch first proj-weight slice during attention
                    wpe_tiles = {}
                    ew = min(512, cfg.C)
                    wpe_tiles[0] = persist.tile([128, cfg.GRP, ew], bf16,
                                                name=f"wpe_0_{rep}", tag="wpe",
                                                bufs=2)
                    nc.scalar.dma_start(wpe_tiles[0][:], wp_d[:, :, 0:ew])
                    for b in range(cfg.B):
                        for tt in range(cfg.NT):
                            tl = slice(tt * 512, (tt + 1) * 512)
                            n_sc = (tt + 1) * 4
                            po = [ops.tile([128, 512], f32, name=f"po_{b}_{tt}_{h}",
                                           tag="po") for h in range(2)]
                            acc = dacc_pool.tile([128, 2, 512], bf16,
                                                 name=f"acc_{b}_{tt}", tag="acc")
                            for sc in range(n_sc):
                                d = sc - tt * 4  # >= 0 on the diagonal
                                sl = slice(sc * 128, (sc + 1) * 128)
                                ph = sps.tile([128, 2, 512], f32,
                                              name=f"ps_{b}_{tt}_{sc}", tag="ps")
                                pr = probs_pool.tile([128, 2, 512], bf16,
                                                     name="pr", tag="pr")
                                if d < 0:
                                    for h in range(2):
                                        nc.tensor.matmul(
                                            ph[:, h, :],
                                            kh_sb[(b, h)][:, sl],
                                            qh_sb[(b, h)][:, tl],
                                            start=True, stop=True)
                                    nc.scalar.activation(pr[:], ph[:], Exp,
                                                         scale=SCALE)
                                else:
                                    c0 = d * 128
                                    for h in range(2):
                                        nc.tensor.matmul(
                                            ph[:, h, c0:512],
                                            kh_sb[(b, h)][:, sl],
                                            qh_sb[(b, h)][:, tt * 512 + c0:
                                                           (tt + 1) * 512],
                                            start=True, stop=True)
                                    nc.scalar.activation(pr[:, :, c0:512],
                                                         ph[:, :, c0:512], Exp,
                                                         scale=SCALE)
                                    # keep j >= p + c0 (t >= s); zero the
                                    # triangle remainder AND the stale cols
                                    # left of the valid range in one pass
                                    nc.gpsimd.affine_select(
                                        pr[:, :, 0:c0 + 128],
                                        pr[:, :, 0:c0 + 128],
                                        pattern=[[0, 2], [1, c0 + 128]],
                                        compare_op=mybir.AluOpType.is_ge,
                                        fill=0.0,
                                        base=-c0,
                                        channel_multiplier=-1)
                                for h in range(2):
                                    nc.tensor.matmul(
                                        po[h][:], v_sb[(b, h)][:, sc, :],
                                        pr[:, h, :],
                                        start=(sc == 0), stop=(sc == n_sc - 1))
                                if sc == 0:
                                    nc.vector.tensor_copy(acc[:], pr[:])
                                else:
                                    nc.vector.tensor_add(acc[:], acc[:], pr[:])
                            pd = sps.tile([128, 2, 512], f32,
                                          name=f"pd_{b}_{tt}", tag="ps")
                            for h in range(2):
                                nc.tensor.matmul(pd[:, h, :], ones_sb[:],
                                                 acc[:, h, :],
                                                 start=True, stop=True)
                            rec = rec_pool.tile([128, 2, 512], f32,
                                                name=f"rec_{b}_{tt}", tag="rec")
                            nc.vector.reciprocal(rec[:], pd[:])
                            for h in range(2):
                                nc.vector.tensor_mul(attn_sb[(b, h)][:, tl],
                                                     po[h][:], rec[:, h, :])

                # ================= Phase D: output projection =================
                with (
                    tc.tile_pool(name=f"ostg_pool{rep}", bufs=4) as ostg_pool,
                    tc.tile_pool(name=f"pps{rep}", bufs=4, space="PSUM") as pps,
                ):
                    for et in range(cfg.ET):
                        el = slice(et * 512, (et + 1) * 512)
                        if et in wpe_tiles:
                            wpe = wpe_tiles[et]
                        else:
                            wpe = persist.tile([128, cfg.GRP, ew], bf16,
                                               name=f"wpe_{et}_{rep}", tag="wpe",
                                               bufs=2)
                            nc.scalar.dma_start(wpe[:], wp_d[:, :, el])
                        for b in range(cfg.B):
                            for hl in range(H_LOCAL):
                                pp = pps.tile([128, ew], f32,
                                              name=f"pp_{et}_{b}_{hl}", tag="pp")
                                at = attn_sb[(b, hl)]
                                for u in range(cfg.GRP):
                                    nc.tensor.matmul(pp[:], at[:, u::cfg.GRP],
                                                     wpe[:, u, :],
                                                     start=(u == 0),
                                                     stop=(u == cfg.GRP - 1))
                                stg = ostg_pool.tile([128, ew], f32,
                                                     name=f"stg_{et}_{b}_{hl}",
                                                     tag="stg")
                                nc.scalar.activation(stg[:], pp[:], Copy)
                                nc.sync.dma_start(out_d[b, hl, :, el], stg[:])

    nc.compile()
    return nc


# =====================================================================
# Host-side input prep / output gather
# =====================================================================

def _part_major(a2d, ncc):
    """[ncc*128, F] -> [128, ncc, F] with row r = chunk*128 + p."""
    F = a2d.shape[1]
    return np.ascontiguousarray(
        a2d.reshape(ncc, 128, F).transpose(1, 0, 2))


def make_trig(cfg: Cfg):
    pos = np.arange(cfg.T, dtype=np.float64)[None, :]        # [1,T]
    j = np.arange(64, dtype=np.float64)[:, None]             # [64,1]
    inv = ROPE_BASE ** (-2.0 * j / Dh)
    ang = pos * inv                                          # [64,T]
    sin = np.sin(ang).astype(np.float32)
    cos = np.cos(ang).astype(np.float32)
    cc2 = np.concatenate([cos, cos], axis=0).astype(BF16)    # [128,T]
    spm = np.concatenate([-sin, sin], axis=0).astype(BF16)
    smp = np.concatenate([sin, -sin], axis=0).astype(BF16)
    return cc2, spm, smp


def make_in_maps(x, w_qkv, w_proj, cfg: Cfg = FULL, n_cores=N_CORES,
                 n_head=N_HEAD):
    x = np.asarray(x, np.float32)
    w_qkv = np.asarray(w_qkv, np.float32)
    w_proj = np.asarray(w_proj, np.float32)
    Cm = cfg.C

    xT = np.ascontiguousarray(x.reshape(cfg.BT, Cm).T)       # [C, BT]
    xt = _part_major(xT, cfg.NCC).astype(BF16)
    wp = _part_major(w_proj, cfg.GRP).astype(BF16)
    cc2, spm, smp = make_trig(cfg)

    wq = w_qkv[:, 0:Cm]
    wk = w_qkv[:, Cm:2 * Cm]
    wv_all = w_qkv[:, 2 * Cm:3 * Cm]

    in_maps = []
    for c in range(n_cores):
        h0, h1 = 2 * c, 2 * c + 1
        q0 = wq[:, h0 * 128:(h0 + 1) * 128]
        q1 = wq[:, h1 * 128:(h1 + 1) * 128]
        k0 = wk[:, h0 * 128:(h0 + 1) * 128]
        k1 = wk[:, h1 * 128:(h1 + 1) * 128]
        qA = np.concatenate([q0[:, 0:64], q1[:, 64:128]], axis=1)
        qB = np.concatenate([q0[:, 64:128], q1[:, 0:64]], axis=1)
        kA = np.concatenate([k0[:, 0:64], k1[:, 64:128]], axis=1)
        kB = np.concatenate([k0[:, 64:128], k1[:, 0:64]], axis=1)
        wqk = _part_major(
            np.concatenate([qA, qB, kA, kB], axis=1), cfg.NCC).astype(BF16)
        wv = _part_major(
            np.concatenate([wv_all[:, h0 * 128:(h0 + 1) * 128],
                            wv_all[:, h1 * 128:(h1 + 1) * 128]], axis=1),
            cfg.NCC).astype(BF16)
        in_maps.append(dict(xt=xt, wqk=wqk, wv=wv, wp=wp,
                            cc2=cc2, spm=spm, smp=smp))
    return in_maps


def gather(outs, cfg: Cfg = FULL):
    """outs: per-core [B, H_LOCAL, 128, C] -> full [B, T, C]."""
    rows = np.concatenate(
        [o.reshape(cfg.B, H_LOCAL * 128, cfg.C) for o in outs], axis=1)
    return np.ascontiguousarray(rows.reshape(cfg.B, cfg.T, cfg.C))


# =====================================================================
# Public entry point
# =====================================================================

_NC_CACHE = {}


def get_nc(debug=False):
    key = ("full", debug)
    if key not in _NC_CACHE:
        _NC_CACHE[key] = build_nc(FULL, debug=debug)
    return _NC_CACHE[key]


def kernel(x, w_qkv, w_proj):
    from concourse.bass_utils import run_bass_kernel_spmd
    nc = get_nc()
    in_maps = make_in_maps(x, w_qkv, w_proj)
    res = run_bass_kernel_spmd(nc, in_maps, list(range(N_CORES)))
    return gather([res.results[c]["out"] for c in range(N_CORES)])

